# revision 7
# baseline (speedup 1.0000x reference)
"""Causal self-attention (B=4, T=2048, HID=768, H=12) on 8 NeuronCores.

Sharding: core c handles batch b=c//2 and head-half c%2 (6 of 12 heads).
Data-parallel on B, tensor-parallel on heads; no cross-device communication.

Per-core kernel (all matmuls fp32r = full-rate fp32):
  - host feeds xT=[768,2048] (hidden[b].T) and W.T column slices so every
    matmul has its contraction dim on SBUF partitions.
  - qT/kT = W.T.T @ xT + b, laid out [128=2 heads x 64d, 2048 tok] per pair,
    so the two heads of a pair run score matmuls concurrently in the PE
    array's two 64-row groups (K=64 row tiling).
  - scores are computed transposed, S^T[k, q], per 128-key chunk; exp on ACT
    (scale=1/8 folded in, no max subtraction -- logits are O(1) by
    construction); causal masking = column-range restriction + triangular
    zeroing of the diagonal 128x128 block via gpsimd affine_select.
  - attention_mask is folded in as exp(s+m) = exp(s)*u, u=exp(m) baked into
    the V tile; V is augmented with a 65th column u so the ctx matmul
    accumulates both ctx_num^T = (u*exp(S))V and the softmax denominator Z
    in one PSUM tile; final division on DVE, broadcast via gpsimd.
  - output is written transposed [384, 2048]; host transposes back.
"""

import sys

for _p in ("/root/.axon_site/_ro/trn_rl_repo", "/opt/trn_rl_repo"):
    if _p not in sys.path:
        sys.path.append(_p)

import numpy as np

import concourse.bass as bass
import concourse.mybir as mybir
import concourse.tile as tile
from concourse import bacc
from concourse.bass_utils import run_bass_kernel_spmd

F32 = mybir.dt.float32
F32R = mybir.dt.float32r

B, T, HID, H = 4, 2048, 768, 12
D = HID // H            # 64
NH = 6                  # heads per core
NPAIR = 3               # head pairs per core
OC = NH * D             # 384 output dims per core
NCI = HID // 128        # 6 contraction chunks
NJ = T // 512           # 4 query chunks of 512
NT16 = T // 128         # 16 token chunks of 128

_TRACE = False
LAST_EXEC_NS = None
_COMPILED = None


def _install_trace_hook():
    import types

    if "antenv.axon_hooks" in sys.modules:
        return
    mod = types.ModuleType("antenv.axon_hooks")
    mod._hook = None
    mod.set_axon_ntff_profile_hook = lambda h: setattr(mod, "_hook", h)
    mod.get_axon_ntff_profile_hook = lambda: mod._hook
    sys.modules["antenv.axon_hooks"] = mod
    sys.path.insert(0, "/root/.axon_site")
    from trn_agent_boot.trn_boot import _ntff_profile_via_ctypes

    mod.set_axon_ntff_profile_hook(
        _ntff_profile_via_ctypes("/opt/axon/libaxon_pjrt.so")
    )


def _build():
    nc = bacc.Bacc("TRN2", target_bir_lowering=False)

    xT = nc.dram_tensor("xT", [HID, T], F32R, kind="ExternalInput")
    wqT = nc.dram_tensor("wqT", [HID, OC], F32R, kind="ExternalInput")
    wkT = nc.dram_tensor("wkT", [HID, OC], F32R, kind="ExternalInput")
    wvT = nc.dram_tensor("wvT", [HID, OC], F32R, kind="ExternalInput")
    bqT = nc.dram_tensor("bqT", [128, NPAIR], F32, kind="ExternalInput")
    bkT = nc.dram_tensor("bkT", [128, NPAIR], F32, kind="ExternalInput")
    bv = nc.dram_tensor("bv", [OC], F32, kind="ExternalInput")
    maskT = nc.dram_tensor("maskT", [128, NT16], F32, kind="ExternalInput")
    outT = nc.dram_tensor("outT", [OC, T], F32, kind="ExternalOutput")

    def r(ap):
        return ap

    with tile.TileContext(nc) as tc:
        consts = tc.alloc_tile_pool(name="consts", bufs=1)
        qk_pool = tc.alloc_tile_pool(name="qk", bufs=1)
        va_pool = tc.alloc_tile_pool(name="va", bufs=1)

        # ---- constants ----
        bq_t = consts.tile([128, NPAIR], F32, tag="bq")
        bk_t = consts.tile([128, NPAIR], F32, tag="bk")
        bvr = consts.tile([128, NH, D], F32, tag="bvr")
        mk_t = consts.tile([128, NT16], F32, tag="mk")
        u_t = consts.tile([128, NT16], F32, tag="u")
        nc.sync.dma_start(out=bq_t, in_=bqT[:, :])
        nc.sync.dma_start(out=bk_t, in_=bkT[:, :])
        nc.gpsimd.dma_start(
            out=bvr,
            in_=bv[:].partition_broadcast(128).rearrange(
                "p (h d) -> p h d", h=NH
            ),
        )
        nc.sync.dma_start(out=mk_t, in_=maskT[:, :])
        nc.scalar.activation(u_t, mk_t, mybir.ActivationFunctionType.Exp)

        # persistent activations
        qT = qk_pool.tile([128, NPAIR, T], F32R, tag="qT")
        kT = qk_pool.tile([128, NPAIR, T], F32R, tag="kT")
        va = va_pool.tile([128, NT16, NH, D + 1], F32R, tag="va")

        if True:
            pin_p = tc.alloc_tile_pool(name="pin", bufs=1)
            xt = pin_p.tile([128, NCI, T], F32R, tag="xt")
            wq_t = pin_p.tile([128, NCI, OC], F32R, tag="wq")
            wk_t = pin_p.tile([128, NCI, OC], F32R, tag="wk")
            wv_t = pin_p.tile([128, NCI, OC], F32R, tag="wv")
            for ci in range(NCI):
                nc.sync.dma_start(out=xt[:, ci, :], in_=xT[128 * ci:128 * (ci + 1), :])
                nc.sync.dma_start(out=wq_t[:, ci, :], in_=wqT[128 * ci:128 * (ci + 1), :])
                nc.sync.dma_start(out=wk_t[:, ci, :], in_=wkT[128 * ci:128 * (ci + 1), :])
                nc.sync.dma_start(out=wv_t[:, ci, :], in_=wvT[128 * ci:128 * (ci + 1), :])

            pps = tc.alloc_tile_pool(name="pps", bufs=2, space="PSUM")

            # ---- q/k projections: out [128 = pair dims, 512 tok] ----
            for w_t, b_t, dst in ((wq_t, bq_t, qT), (wk_t, bk_t, kT)):
                for pi in range(NPAIR):
                    for tj in range(NJ):
                        ps = pps.tile([128, 512], F32, tag="ps")
                        for ci in range(NCI):
                            nc.tensor.matmul(
                                ps,
                                r(w_t[:, ci, 128 * pi:128 * (pi + 1)]),
                                r(xt[:, ci, 512 * tj:512 * (tj + 1)]),
                                start=(ci == 0),
                                stop=(ci == NCI - 1),
                            )
                        nc.vector.tensor_scalar_add(
                            dst[:, pi, 512 * tj:512 * (tj + 1)], ps,
                            b_t[:, pi:pi + 1],
                        )

            # ---- v projection: out [128 tok, 384 dims] + u augmentation ----
            for t16 in range(NT16):
                ps = pps.tile([128, OC], F32, tag="ps")
                for ci in range(NCI):
                    nc.tensor.matmul(
                        ps,
                        r(xt[:, ci, 128 * t16:128 * (t16 + 1)]),
                        r(wv_t[:, ci, :]),
                        start=(ci == 0),
                        stop=(ci == NCI - 1),
                    )
                va_t = va[:, t16]                    # [128, NH, D+1]
                ps_r = ps.rearrange("p (h d) -> p h d", h=NH)
                uc = u_t[:, t16:t16 + 1]
                nc.vector.tensor_tensor(
                    va_t[:, :, 0:D], ps_r, bvr, op=mybir.AluOpType.add
                )
                nc.vector.tensor_scalar_mul(va_t[:, :, 0:D], va_t[:, :, 0:D], uc)
                nc.vector.tensor_copy(va_t[:, :, D], uc.to_broadcast([128, NH]))

        # ---- attention ----
        pps.release()
        pin_p.release()
        sp = tc.alloc_tile_pool(name="sp", bufs=4, space="PSUM")
        cxa = tc.alloc_tile_pool(name="cxa", bufs=1, space="PSUM")
        cxb = tc.alloc_tile_pool(name="cxb", bufs=1, space="PSUM")
        pt_pool = tc.alloc_tile_pool(name="pt", bufs=6)
        npool = tc.alloc_tile_pool(name="np", bufs=3)

        for pi in range(NPAIR):
            for j in range(NJ):
                nk = 4 * (j + 1)
                ctx = (cxa.tile([D + 1, 512], F32, tag="cA", name="ctxA"),
                       cxb.tile([D + 1, 512], F32, tag="cB", name="ctxB"))
                pend = []
                for kc in range(nk):
                    c0 = max(0, kc - 4 * j) * 128
                    pts = []
                    for half in range(2):
                        rows = slice(64 * half, 64 * half + 64)
                        s_ps = sp.tile([128, 512], F32, tag="s")
                        nc.tensor.matmul(
                            s_ps[:, c0:],
                            r(kT[rows, pi, 128 * kc:128 * (kc + 1)]),
                            r(qT[rows, pi, 512 * j + c0:512 * (j + 1)]),
                            start=True, stop=True,
                        )
                        pt = pt_pool.tile([128, 512], F32R, tag="pt")
                        nc.scalar.activation(
                            pt[:, c0:], s_ps[:, c0:],
                            mybir.ActivationFunctionType.Exp, scale=0.125,
                        )
                        if kc >= 4 * j:  # diagonal chunk: zero below-diagonal
                            nc.gpsimd.affine_select(
                                out=pt[:, c0:c0 + 128],
                                in_=pt[:, c0:c0 + 128],
                                compare_op=mybir.AluOpType.is_ge,
                                fill=0.0,
                                base=0,
                                pattern=[[1, 128]],
                                channel_multiplier=-1,
                            )
                        pts.append(pt)
                    pend.append((kc, c0, pts))
                    if len(pend) > 1:
                        _emit_ctx(nc, r, va, ctx, pend.pop(0), pi, nk)
                _emit_ctx(nc, r, va, ctx, pend.pop(0), pi, nk)

                for half in range(2):
                    hl = 2 * pi + half
                    zrow = npool.tile([1, 512], F32, tag="zrow")
                    nc.vector.tensor_copy(zrow, ctx[half][D:D + 1, :])
                    zrep = npool.tile([64, 512], F32, tag="zrep")
                    nc.gpsimd.partition_broadcast(zrep, zrow)
                    rrep = npool.tile([64, 512], F32, tag="rrep")
                    nc.vector.reciprocal(rrep, zrep)
                    ot = npool.tile([64, 512], F32, tag="ot")
                    nc.vector.tensor_tensor(
                        ot, ctx[half][0:D, :], rrep, op=mybir.AluOpType.mult
                    )
                    nc.sync.dma_start(
                        out=outT[D * hl:D * (hl + 1), 512 * j:512 * (j + 1)],
                        in_=ot,
                    )

        npool.release()
        pt_pool.release()
        cxb.release()
        cxa.release()
        sp.release()
        va_pool.release()
        qk_pool.release()
        consts.release()

    nc.compile()
    return nc


def _emit_ctx(nc, r, va, ctx, item, pi, nk):
    kc, c0, pts = item
    for half in range(2):
        hl = 2 * pi + half
        nc.tensor.matmul(
            ctx[half][:, c0:],
            r(va[:, kc, hl, :]),
            r(pts[half][:, c0:]),
            start=(kc == 0),
            stop=(kc == nk - 1),
        )


def kernel(**inputs):
    global _COMPILED, LAST_EXEC_NS
    hs = np.asarray(inputs["hidden_states"], dtype=np.float32)
    am = np.asarray(inputs["attention_mask"], dtype=np.float32)
    Wq = np.asarray(inputs["Wq"], dtype=np.float32)
    bq = np.asarray(inputs["bq"], dtype=np.float32)
    Wk = np.asarray(inputs["Wk"], dtype=np.float32)
    bk = np.asarray(inputs["bk"], dtype=np.float32)
    Wv = np.asarray(inputs["Wv"], dtype=np.float32)
    bv = np.asarray(inputs["bv"], dtype=np.float32)

    if _COMPILED is None:
        _COMPILED = _build()
    nc = _COMPILED

    c = np.ascontiguousarray
    in_maps = []
    for core in range(8):
        b, half = core // 2, core % 2
        o0 = OC * half
        sl = slice(o0, o0 + OC)
        in_maps.append({
            "xT": c(hs[b].T),                                  # [768, 2048]
            "wqT": c(Wq[sl, :].T),                             # [768, 384]
            "wkT": c(Wk[sl, :].T),
            "wvT": c(Wv[sl, :].T),
            "bqT": c(bq[sl].reshape(NPAIR, 128).T),
            "bkT": c(bk[sl].reshape(NPAIR, 128).T),
            "bv": c(bv[sl]),
            "maskT": c(am[b, 0, 0, :].reshape(NT16, 128).T),
        })

    if _TRACE:
        _install_trace_hook()
    res = run_bass_kernel_spmd(nc, in_maps, list(range(8)), trace=_TRACE)
    LAST_EXEC_NS = res.exec_time_ns

    out = np.empty((B, T, HID), dtype=np.float32)
    for core in range(8):
        b, half = core // 2, core % 2
        out[b, :, OC * half:OC * (half + 1)] = res.results[core]["outT"].T
    return out
